# revision 25
# baseline (speedup 1.0000x reference)
"""Causal self-attention (RoPE) Trainium2 kernel, 8-core SPMD.

Sharding:
  Launch A: tensor-parallel over heads — core i computes heads (2i, 2i+1):
    qkv projection slice, RoPE, causal flash-style attention. Ships
    unnormalized y (bf16) + softmax denominators.
  Host: normalize y by denominators, reshuffle head-shards -> token-shards.
  Launch B: data-parallel over tokens — core i projects its 512 token rows
    through the full w_proj (+bias) -> out [512, 1024].

All matmul inputs are bf16 (fp32 matmuls trigger a 50% PE utilization
throttle on trn2); PSUM accumulation is fp32. Softmax is computed without
max-subtraction (scores ~ N(0,1) after the 1/sqrt(D) scale) and masking is
applied on the exp'd probabilities (zeroing) via gpsimd affine_select,
which is mathematically identical to masking the scores with -inf.

Layout notes:
  - q/k are kept "transposed" (head dims on partitions, tokens free), with
    head dims permuted to [evens | odds] so RoPE's rotate-half becomes a
    32-partition block swap. v is computed directly token-major
    (lhsT = x block), with a ones-column appended per head so the AV matmul
    also produces the softmax denominator for free.
  - Scores are computed transposed ([k, q]) so the AV matmul consumes probs
    directly. exp() runs on 2-key-block-wide PSUM regions to amortize the
    scalar-engine per-instruction overhead.
"""

import numpy as np
from contextlib import ExitStack

import ml_dtypes
import concourse.bass as bass
import concourse.tile as tile
from concourse import bacc, mybir
from concourse.bass_utils import run_bass_kernel_spmd

F32 = mybir.dt.float32
BF16 = mybir.dt.bfloat16
NPBF = ml_dtypes.bfloat16

B, T, C = 2, 2048, 1024
H, D = 16, 64
NCORES = 8
HPC = H // NCORES          # heads per core (2)
BT = B * T                 # 4096 token rows
TCHUNK = 512               # token chunk (matmul moving size)
NKT_C = C // 128           # k-tiles over the C contraction (8)
NTCH = BT // TCHUNK        # token chunks (8)


def build_launch_a():
    nc = bacc.Bacc("TRN2", target_bir_lowering=False, debug=False,
                   num_devices=NCORES)
    xT_d = nc.dram_tensor("xT", [C, BT], BF16, kind="ExternalInput").ap()
    wq_d = nc.dram_tensor("wq", [C, 128], BF16, kind="ExternalInput").ap()
    wk_d = nc.dram_tensor("wk", [C, 128], BF16, kind="ExternalInput").ap()
    wv_d = nc.dram_tensor("wv", [C, 128], BF16, kind="ExternalInput").ap()
    cos_d = nc.dram_tensor("cosT", [128, T], BF16, kind="ExternalInput").ap()
    sin_d = nc.dram_tensor("sinT", [128, T], BF16, kind="ExternalInput").ap()
    # rows [65h, 65h+64] = head h dims; row 65h+64 = head h softmax denom
    y_d = nc.dram_tensor("y", [130, BT], BF16, kind="ExternalOutput").ap()

    with tile.TileContext(nc) as tc, ExitStack() as ctx:
        consts = ctx.enter_context(tc.tile_pool(name="consts", bufs=1))
        persist = ctx.enter_context(tc.tile_pool(name="persist", bufs=1))
        xin = ctx.enter_context(tc.tile_pool(name="xin", bufs=2))
        work = ctx.enter_context(tc.tile_pool(name="work", bufs=3))
        probp = ctx.enter_context(tc.tile_pool(name="probp", bufs=4))
        psproj = ctx.enter_context(tc.tile_pool(name="psproj", bufs=1, space="PSUM"))
        psv = ctx.enter_context(tc.tile_pool(name="psv", bufs=1, space="PSUM"))
        # 1 tag x 2 bufs x 2 banks each = 4 PSUM banks (2-deep kt pipeline)
        psscore = ctx.enter_context(tc.tile_pool(name="psscore", bufs=2, space="PSUM"))
        psy = ctx.enter_context(tc.tile_pool(name="psy", bufs=1, space="PSUM"))

        # --- constants --- (wq first: the first projection only needs wq +
        # x chunk 0, so don't queue 1.75MB of other constants ahead of it)
        wq_t = consts.tile([128, NKT_C, 128], BF16, tag="wq")
        wk_t = consts.tile([128, NKT_C, 128], BF16, tag="wk")
        wv_t = consts.tile([128, NKT_C, 128], BF16, tag="wv")
        cos_t = consts.tile([128, T], BF16, tag="cos")
        sin_t = consts.tile([128, T], BF16, tag="sin")
        nc.sync.dma_start(out=wq_t[:], in_=wq_d.rearrange("(kt p) c -> p kt c", p=128))
        nc.scalar.dma_start(out=wv_t[:], in_=wv_d.rearrange("(kt p) c -> p kt c", p=128))
        nc.scalar.dma_start(out=wk_t[:], in_=wk_d.rearrange("(kt p) c -> p kt c", p=128))
        nc.scalar.dma_start(out=cos_t[:], in_=cos_d)
        nc.scalar.dma_start(out=sin_t[:], in_=sin_d)

        # --- HAM pre-warm: ~3.4us of dummy matmuls while the first input
        # DMAs are in flight, so real matmuls start at 2.4 GHz instead of
        # the cold 1.2 GHz default ---
        scr = consts.tile([128, 256], BF16, tag="scr")
        nc.gpsimd.memset(scr[:], 0.0)
        warm = psproj.tile([128, TCHUNK], F32, tag="psproj", name="warm")
        for i in range(16):
            # one accumulation group -> back-to-back issue (no WAW stalls)
            nc.tensor.matmul(warm[:, 0:256], scr[:, 0:128], scr[:],
                             start=(i == 0), stop=(i == 15))

        # --- persistent intermediates ---
        qrot = persist.tile([128, BT], BF16, tag="qrot")
        krot = persist.tile([128, BT], BF16, tag="krot")
        # v in natural [token, dim] layout per global 128-token tile, columns
        # [h0 dims | ones | h1 dims | ones] so each head's AV lhsT is a
        # contiguous [128, 65] slice whose last column computes the softmax
        # denominator for free.
        vnat = persist.tile([128, BT // 128, 130], BF16, tag="vnat")
        nc.gpsimd.memset(vnat[:, :, 64:65], 1.0)
        nc.gpsimd.memset(vnat[:, :, 129:130], 1.0)

        # =================== phase 1: projections + RoPE ===================
        for tch in range(NTCH):
            g0 = tch * TCHUNK
            bb, t0 = divmod(g0, T)
            xt = xin.tile([128, NKT_C, TCHUNK], BF16, tag="xt")
            xsrc = xT_d.rearrange("(kt p) t -> p kt t", p=128)[:, :, g0:g0 + TCHUNK]
            if tch == 0:
                # split the first chunk so the first projection matmuls
                # start after half the transfer
                nc.sync.dma_start(out=xt[:, 0:4, :], in_=xsrc[:, 0:4, :])
                nc.sync.dma_start(out=xt[:, 4:8, :], in_=xsrc[:, 4:8, :])
            else:
                nc.sync.dma_start(out=xt[:], in_=xsrc)

            # q and k share one [128, 2, 512] staging tile so the rotate-half
            # partition swap is 4 DMAs per chunk instead of 8. Emission order
            # q -> v -> k lets the v matmuls cover the q PSUM-copy latency
            # (psproj has a single buffer).
            raw2 = work.tile([128, 2, TCHUNK], BF16, tag="raw2")
            sh2 = work.tile([128, 2, TCHUNK], BF16, tag="sh2")
            psq = psproj.tile([128, TCHUNK], F32, tag="psproj", name="psq")
            for kt in range(NKT_C):
                nc.tensor.matmul(psq[:], wq_t[:, kt, :], xt[:, kt, :],
                                 start=(kt == 0), stop=(kt == NKT_C - 1))
            nc.vector.tensor_copy(raw2[:, 0, :], psq[:])

            # v directly in token-major layout: out[token, dim] with
            # lhsT = x block (tokens as PE columns), rhs = wv.
            psv_t = psv.tile([128, 4, 128], F32, tag="psv")
            for j in range(4):
                for kt in range(NKT_C):
                    nc.tensor.matmul(
                        psv_t[:, j, :],
                        xt[:, kt, 128 * j:128 * (j + 1)],
                        wv_t[:, kt, :],
                        start=(kt == 0), stop=(kt == NKT_C - 1))
            ktg0 = g0 // 128
            nc.vector.tensor_copy(vnat[:, ktg0:ktg0 + 4, 0:64],
                                  psv_t[:, :, 0:64])
            nc.vector.tensor_copy(vnat[:, ktg0:ktg0 + 4, 65:129],
                                  psv_t[:, :, 64:128])

            psk = psproj.tile([128, TCHUNK], F32, tag="psproj", name="psk")
            for kt in range(NKT_C):
                nc.tensor.matmul(psk[:], wk_t[:, kt, :], xt[:, kt, :],
                                 start=(kt == 0), stop=(kt == NKT_C - 1))
            nc.vector.tensor_copy(raw2[:, 1, :], psk[:])

            # rotate-half: swap 32-row blocks within each head (q + k at once)
            for blk in range(4):
                src = blk ^ 1
                nc.sync.dma_start(out=sh2[32 * blk:32 * (blk + 1), :, :],
                                  in_=raw2[32 * src:32 * (src + 1), :, :])
            cslice = cos_t[:, t0:t0 + TCHUNK]
            sslice = sin_t[:, t0:t0 + TCHUNK]
            for idx, dest in ((0, qrot), (1, krot)):
                tmp = work.tile([128, TCHUNK], BF16, tag="ropetmp")
                nc.vector.tensor_mul(dest[:, g0:g0 + TCHUNK],
                                     raw2[:, idx, :], cslice)
                nc.vector.tensor_mul(tmp[:], sh2[:, idx, :], sslice)
                nc.vector.tensor_add(dest[:, g0:g0 + TCHUNK],
                                     dest[:, g0:g0 + TCHUNK], tmp[:])

        # =================== phase 2: attention ===================
        # High priority: the attention stream (ACT-paced exp chain) is the
        # critical path; phase-1 projection matmuls act as PE gap-filler so
        # the HAM clock gate stays warm through the endgame.
        ctx.enter_context(tc.high_priority(offset=1_000_000))
        scale = float(1.0 / np.sqrt(D))
        for bb in range(B):
            for qc in range(T // TCHUNK):
                q0 = qc * TCHUNK
                gq = bb * T + q0
                nkt = (q0 + TCHUNK) // 128
                ys = [psy.tile([65, TCHUNK], F32, tag=f"psy{h}", name=f"psy{h}")
                      for h in range(HPC)]
                for kt in range(nkt):
                    k0 = 128 * kt
                    j0 = k0 - q0
                    js = max(j0, 0)
                    ktg = (bb * T + k0) // 128
                    # both heads share one [128, 2, 512] PSUM tile so their
                    # score matmuls become ready together: adjacent in the
                    # PE queue with disjoint row groups (rows 0-63 vs
                    # 64-127), they execute concurrently in the array. One
                    # exp covers both heads.
                    pss = psscore.tile([128, 2, TCHUNK], F32, tag="pss")
                    for h in range(HPC):
                        hp = 64 * h
                        nc.tensor.matmul(
                            pss[:, h, js:TCHUNK],
                            krot[hp:hp + 64, bb * T + k0:bb * T + k0 + 128],
                            qrot[hp:hp + 64, gq + js:gq + TCHUNK],
                            start=True, stop=True, tile_position=(hp, 0))
                    probs = probp.tile([128, 2, TCHUNK], BF16, tag="probs")
                    nc.scalar.activation(
                        probs[:, :, js:TCHUNK], pss[:, :, js:TCHUNK],
                        mybir.ActivationFunctionType.Exp, scale=scale)
                    if j0 >= 0:
                        # diagonal block: zero probs where key > query
                        # (same predicate for both heads)
                        nc.gpsimd.affine_select(
                            out=probs[:, :, j0:j0 + 128],
                            in_=probs[:, :, j0:j0 + 128],
                            pattern=[[0, 2], [1, 128]],
                            compare_op=mybir.AluOpType.is_ge,
                            fill=0.0,
                            base=0,
                            channel_multiplier=-1)
                    for h in range(HPC):
                        nc.tensor.matmul(ys[h][:, js:TCHUNK],
                                         vnat[:, ktg, 65 * h:65 * h + 65],
                                         probs[:, h, js:TCHUNK],
                                         start=(kt == 0), stop=(kt == nkt - 1))
                # epilogue: ship unnormalized y + denominators (row 64)
                for h in range(HPC):
                    yts = work.tile([65, TCHUNK], BF16, tag="yts")
                    nc.vector.tensor_copy(yts[:], ys[h][:])
                    nc.sync.dma_start(out=y_d[65 * h:65 * h + 65, gq:gq + TCHUNK],
                                      in_=yts[:])

    nc.compile()
    return nc


def build_launch_b():
    nc = bacc.Bacc("TRN2", target_bir_lowering=False, debug=False,
                   num_devices=NCORES)
    TLOC = BT // NCORES  # 512 tokens per core
    yT_d = nc.dram_tensor("yT", [C, TLOC], BF16, kind="ExternalInput").ap()
    wp_d = nc.dram_tensor("wp", [C, C], BF16, kind="ExternalInput").ap()
    out_d = nc.dram_tensor("out", [TLOC, C], F32, kind="ExternalOutput").ap()

    with tile.TileContext(nc) as tc, ExitStack() as ctx:
        consts = ctx.enter_context(tc.tile_pool(name="consts", bufs=1))
        pspool = ctx.enter_context(tc.tile_pool(name="ps", bufs=1, space="PSUM"))

        wp_t = consts.tile([128, NKT_C, C], BF16, tag="wp")
        yT_t = consts.tile([128, NKT_C, TLOC], BF16, tag="yT")
        # per-kt input DMAs so kt=0 matmuls start after ~384KB instead of
        # waiting for the full 3MB load (in-order PE queue).
        for kt in range(NKT_C):
            nc.sync.dma_start(out=yT_t[:, kt, :],
                              in_=yT_d[128 * kt:128 * (kt + 1), :])
            nc.sync.dma_start(out=wp_t[:, kt, :],
                              in_=wp_d[128 * kt:128 * (kt + 1), :])

        # HAM pre-warm while input DMAs are in flight
        scr = consts.tile([128, 256], BF16, tag="scr")
        nc.gpsimd.memset(scr[:], 0.0)
        warm = pspool.tile([128, TCHUNK], F32, tag="ps00", name="warm")
        for i in range(16):
            # one accumulation group -> back-to-back issue (no WAW stalls)
            nc.tensor.matmul(warm[:, 0:256], scr[:, 0:128], scr[:],
                             start=(i == 0), stop=(i == 15))

        # kt-outer accumulation into 8 persistent PSUM banks (one per
        # (mt, nch) output tile) so compute streams behind the input DMAs.
        # Token rows are processed in two halves so the first half's output
        # drain (PSUM->SBUF copy + DMA) overlaps the second half's matmuls.
        # The bias is added on the host.
        pss = {}
        for mt in range(TLOC // 128):
            for nch in range(C // TCHUNK):
                pss[(mt, nch)] = pspool.tile([128, TCHUNK], F32,
                                             tag=f"ps{mt}{nch}",
                                             name=f"ps{mt}{nch}")
        work = ctx.enter_context(tc.tile_pool(name="work", bufs=4))
        for half in range(2):
            mts = (2 * half, 2 * half + 1)
            for kt in range(NKT_C):
                for mt in mts:                       # 2 output row tiles
                    for nch in range(C // TCHUNK):   # 2 output col chunks
                        nc.tensor.matmul(
                            pss[(mt, nch)][:],
                            yT_t[:, kt, 128 * mt:128 * (mt + 1)],
                            wp_t[:, kt, TCHUNK * nch:TCHUNK * (nch + 1)],
                            start=(kt == 0), stop=(kt == NKT_C - 1))
            for mt in mts:
                for nch in range(C // TCHUNK):
                    ot = work.tile([128, TCHUNK], F32, tag="ot", name="ot")
                    # split the PSUM->SBUF drain across vector and scalar,
                    # and the DMA issues across the two HWDGE queues
                    if (2 * mt + nch) % 2 == 0:
                        nc.vector.tensor_copy(ot[:], pss[(mt, nch)][:])
                        eng = nc.sync
                    else:
                        nc.scalar.copy(ot[:], pss[(mt, nch)][:])
                        eng = nc.scalar
                    eng.dma_start(
                        out=out_d[128 * mt:128 * (mt + 1),
                                  TCHUNK * nch:TCHUNK * (nch + 1)],
                        in_=ot[:])

    nc.compile()
    return nc


def _host_prep(x, w_qkv):
    xT = np.ascontiguousarray(x.reshape(BT, C).T.astype(NPBF))  # [C, BT] bf16
    perm = np.concatenate([np.arange(0, D, 2), np.arange(1, D, 2)])
    inv = 1.0 / (10000.0 ** (np.arange(0, D, 2, dtype=np.float64) / D))  # [32]
    f = np.outer(np.arange(T, dtype=np.float64), inv)  # [T, 32]
    cosT = np.cos(f).T.astype(np.float32)  # [32, T]
    sinT = np.sin(f).T.astype(np.float32)
    c64 = np.concatenate([cosT, cosT], 0)
    s64 = np.concatenate([-sinT, sinT], 0)
    C128 = np.ascontiguousarray(np.concatenate([c64, c64], 0).astype(NPBF))
    S128 = np.ascontiguousarray(np.concatenate([s64, s64], 0).astype(NPBF))

    in_maps = []
    for i in range(NCORES):
        h0, h1 = HPC * i, HPC * i + 1
        wq = np.concatenate([w_qkv[:, h0 * D + perm], w_qkv[:, h1 * D + perm]], 1)
        wk = np.concatenate([w_qkv[:, C + h0 * D + perm], w_qkv[:, C + h1 * D + perm]], 1)
        wv = np.concatenate([w_qkv[:, 2 * C + h0 * D:2 * C + (h0 + 1) * D],
                             w_qkv[:, 2 * C + h1 * D:2 * C + (h1 + 1) * D]], 1)
        in_maps.append({
            "xT": xT,
            "wq": np.ascontiguousarray(wq.astype(NPBF)),
            "wk": np.ascontiguousarray(wk.astype(NPBF)),
            "wv": np.ascontiguousarray(wv.astype(NPBF)),
            "cosT": C128, "sinT": S128,
        })
    return in_maps


_CACHE = {}


def _get_kernels():
    if "a" not in _CACHE:
        _CACHE["a"] = build_launch_a()
        _CACHE["b"] = build_launch_b()
    return _CACHE["a"], _CACHE["b"]


def run(x, w_qkv, w_proj, b_proj, trace=False, tmpdirs=(None, None)):
    nca, ncb = _get_kernels()
    in_maps_a = _host_prep(np.asarray(x), np.asarray(w_qkv))
    res_a = run_bass_kernel_spmd(nca, in_maps_a, list(range(NCORES)),
                                 trace=trace, tmpdir=tmpdirs[0])
    # normalize + assemble yT_full[h*64+d, token], then shard by token chunk
    yT = np.empty((C, BT), dtype=np.float32)
    for i in range(NCORES):
        ya = np.asarray(res_a.results[i]["y"], dtype=np.float32)  # [130, 4096]
        for h in range(HPC):
            gh = HPC * i + h
            yT[64 * gh:64 * (gh + 1)] = (ya[65 * h:65 * h + 64]
                                         / ya[65 * h + 64][None, :])
    wp = np.ascontiguousarray(np.asarray(w_proj).astype(NPBF))
    TLOC = BT // NCORES
    in_maps_b = [{
        "yT": np.ascontiguousarray(yT[:, i * TLOC:(i + 1) * TLOC].astype(NPBF)),
        "wp": wp,
    } for i in range(NCORES)]
    res_b = run_bass_kernel_spmd(ncb, in_maps_b, list(range(NCORES)),
                                 trace=trace, tmpdir=tmpdirs[1])
    out = np.concatenate([res_b.results[i]["out"] for i in range(NCORES)], 0)
    out += np.asarray(b_proj, dtype=np.float32)[None, :]
    return out.reshape(B, T, C).astype(np.float32), res_a, res_b


def kernel(x, w_qkv, w_proj, b_proj):
    out, _, _ = run(x, w_qkv, w_proj, b_proj)
    return out
